# revision 53
# baseline (speedup 1.0000x reference)
"""Trainium2 Bass kernel for LocalScopeSelfAttention (3x3 window, clamp-padded).

Shapes (hardcoded): x [2, 8, 32, 32, 256] f32, 8 heads x hd=32, LN eps 1e-5.
Sharding: data-parallel over B*T=16 frames -> 2 frames per core on 8 cores.

v2: tensor-engine pipelining. The PE runs at stream rate (0.42ns/col) when the
instruction queue is free of dependency stalls, so the kernel is organized to
keep the tensor queue back-to-back:
  - all per-frame tensors are duplicated (frame-indexed) so frame f+1's
    preamble overlaps frame f's attention tail
  - the clamp-multiplicity mask is applied as a log-mask ACCUMULATING MATMUL
    into the scores psum (exp(s + ln m) == m * exp(s)), removing the
    elementwise mask multiply entirely
  - attention is software-pipelined: AV matmuls lag the scores matmuls by two
    subtiles so the scalar-engine exp has time to land
  - subtiles are paired in the AV psum ([128, 8, 33], odd subtile in
    partitions 64:128) halving the normalize/transpose work downstream
"""

import numpy as np
import ml_dtypes

H = W = 32
N = H * W          # 1024 tokens per frame
D = 256
NH, HD = 8, 32
LN_EPS = 1e-5
N_CORES = 8
FPC = 2            # frames per core
NPAD = N + 64      # padded tokens (32 guard each side)

_COMPILED = None


# ---------------------------------------------------------------- host helpers
def _build_lnmask_np():
    colcount = np.zeros((W, W), np.float32)
    for qc in range(W):
        for dc in (-1, 0, 1):
            colcount[qc, min(max(qc + dc, 0), W - 1)] += 1
    # rowcount[v][rq, rp] ; window rows are 2s-1 .. 2s+2 (rp = row - (2s-1))
    rowcounts = np.zeros((3, 2, 4), np.float32)
    for v, s in ((0, 0), (1, 7), (2, 15)):
        for rq in (0, 1):
            for dh in (-1, 0, 1):
                tgt = min(max(2 * s + rq + dh, 0), H - 1)
                rowcounts[v, rq, tgt - (2 * s - 1)] += 1
    masks = np.zeros((128, 3, 64), np.float32)
    for p in range(128):
        rp, kc = p // 32, p % 32
        for j in range(64):
            rq, qc = j // 32, j % 32
            for v in range(3):
                masks[p, v, j] = rowcounts[v, rq, rp] * colcount[qc, kc]
    lnm = np.where(masks > 0, np.log(np.maximum(masks, 1e-6)), -80.0)
    # repeat the [128, 64] block for all 8 (Q, g) head slots -> [128, 3, 512]
    lnm = np.tile(lnm[:, :, None, :], (1, 1, 8, 1)).reshape(128, 3, 512)
    # fp8e4m3 halves the DMA: values are {0, log2, log4, -80}; log2 rounds to
    # 0.6875 (exp -> 1.989 instead of 2, a 0.6% error on clamp-doubled
    # weights) which is far inside the error budget
    return lnm.astype(ml_dtypes.float8_e4m3)


def _fold_params(inp):
    f32 = np.float32
    g = inp["ln_g"].astype(f32)
    lb = inp["ln_b"].astype(f32)
    s = f32(1.0 / np.sqrt(HD))
    wq = (g[:, None] * inp["wq"].astype(f32)) * s
    bq = (lb @ inp["wq"].astype(f32) + inp["bq"].astype(f32)) * s
    wk = g[:, None] * inp["wk"].astype(f32)
    wv = g[:, None] * inp["wv"].astype(f32)
    bv = lb @ inp["wv"].astype(f32) + inp["bv"].astype(f32)
    wo = inp["wo"].astype(f32)
    bo = bv @ wo + inp["bo"].astype(f32)
    bf = ml_dtypes.bfloat16
    # weight sbuf layout [128, kc, m]: w[kc*128+p, m]
    def wfmt(w):
        return np.ascontiguousarray(w.reshape(2, 128, 256).transpose(1, 0, 2)).astype(bf)
    return {
        "wq": wfmt(wq), "wk": wfmt(wk), "wv": wfmt(wv), "wo": wfmt(wo),
        # bq as [128, 2] f32: per-partition bias for the two 128-dim out blocks
        "bqp": np.ascontiguousarray(bq.reshape(2, 128).T).astype(f32),
        # bo broadcast to all 128 partitions: added into the residual x_f
        "bob": np.ascontiguousarray(np.tile(bo.reshape(1, 256), (128, 1))).astype(bf),
        "lnm": _build_lnmask_np(),
    }


# ---------------------------------------------------------------- bass build
def _build_bass():
    from contextlib import ExitStack
    import concourse.tile as tile
    from concourse import bacc, mybir

    dt = mybir.dt
    AF = mybir.ActivationFunctionType
    OP = mybir.AluOpType

    nc = bacc.Bacc("TRN2", target_bir_lowering=False, debug=False,
                   num_devices=N_CORES)

    x_d = nc.dram_tensor("x", [FPC * N, D], dt.bfloat16, kind="ExternalInput").ap()
    wq_d = nc.dram_tensor("wq", [128, 2, 256], dt.bfloat16, kind="ExternalInput").ap()
    wk_d = nc.dram_tensor("wk", [128, 2, 256], dt.bfloat16, kind="ExternalInput").ap()
    wv_d = nc.dram_tensor("wv", [128, 2, 256], dt.bfloat16, kind="ExternalInput").ap()
    wo_d = nc.dram_tensor("wo", [128, 2, 256], dt.bfloat16, kind="ExternalInput").ap()
    bqp_d = nc.dram_tensor("bqp", [128, 2], dt.float32, kind="ExternalInput").ap()
    bob_d = nc.dram_tensor("bob", [128, 256], dt.bfloat16, kind="ExternalInput").ap()
    lnm_d = nc.dram_tensor("lnm", [128, 3, 512], dt.float8e4, kind="ExternalInput").ap()
    y_d = nc.dram_tensor("y", [FPC * N, D], dt.bfloat16, kind="ExternalOutput").ap()

    with tile.TileContext(nc) as tc:
        with ExitStack() as ctx:
            const = ctx.enter_context(tc.tile_pool(name="const", bufs=1))
            frame = ctx.enter_context(tc.tile_pool(name="frame", bufs=1))
            work = ctx.enter_context(tc.tile_pool(name="work", bufs=3))
            att = ctx.enter_context(tc.tile_pool(name="att", bufs=4))
            pp = ctx.enter_context(tc.tile_pool(name="pp", bufs=2, space="PSUM"))
            pst = ctx.enter_context(tc.tile_pool(name="pst", bufs=3, space="PSUM"))
            pav = ctx.enter_context(tc.tile_pool(name="pav", bufs=2, space="PSUM"))
            ptp = ctx.enter_context(tc.tile_pool(name="ptp", bufs=1, space="PSUM"))

            # ---- constants ----
            # identity first: it is built on-chip (gpsimd) and unblocks the
            # PE warm-up fillers without waiting for any DMA
            ident = const.tile([128, 128], dt.bfloat16)
            from concourse.masks import make_identity
            make_identity(nc, ident[:])
            # fp8 identity: stationary operand for the fp8 log-mask matmul
            ident8 = const.tile([128, 128], dt.float8e4)
            make_identity(nc, ident8[:])
            for cval in (0.0, LN_EPS):
                ct = const.tile([128, 1], dt.float32, tag=f"c{cval}")
                nc.vector.memset(ct[:], cval)
                nc.const_aps.aps[(dt.float32, cval)] = ct[:]
            wq_s = const.tile([128, 2, 256], dt.bfloat16)
            wk_s = const.tile([128, 2, 256], dt.bfloat16)
            wv_s = const.tile([128, 2, 256], dt.bfloat16)
            wo_s = const.tile([128, 2, 256], dt.bfloat16)
            bqp_s = const.tile([128, 2], dt.float32)
            bob_s = const.tile([128, 256], dt.bfloat16)
            lnm_s = const.tile([128, 3, 512], dt.float8e4)
            # weight DMAs first (needed ~10us in); bqp/bob/lnm are emitted
            # mid-preamble so their descriptor-gen does not block the scalar
            # engine's first Sqrt (the LN critical path)
            for sb, d in ((wq_s, wq_d), (wk_s, wk_d), (wv_s, wv_d), (wo_s, wo_d)):
                nc.scalar.dma_start(sb[:], d[:])

            # ---- per-frame persistent tensors, duplicated over FPC ----
            def ftiles(shape, dtype, nm):
                return [frame.tile(shape, dtype, name=f"{nm}{f}")
                        for f in range(FPC)]

            xnT = ftiles([128, 2, NPAD], dt.bfloat16, "xnT")
            kTp = ftiles([128, 2, NPAD], dt.bfloat16, "kTp")
            qst = frame.tile([128, 2, 4, N], dt.bfloat16, name="qst")
            vau = ftiles([128, 9, NH, 33], dt.bfloat16, "vau")
            vau64 = ftiles([128, 8, NH, 33], dt.bfloat16, "vau64")
            xoT = ftiles([128, 2, N], dt.bfloat16, "xoT")
            x_f = ftiles([128, 8, 256], dt.bfloat16, "x_f")
            mv = ftiles([128, 8, 2], dt.float32, "mv")
            rstd = ftiles([128, 8], dt.float32, "rstd")
            lnv = ftiles([128, 8], dt.float32, "lnv")

            # qst is shared by both frames: stripes are rewritten per frame,
            # the inter-stripe zeros are written once here. Both halves go to
            # gpsimd (it is idle at start) so the vector engine can begin the
            # LayerNorm as soon as the first x chunk lands.
            nc.gpsimd.memset(qst[:, 0], 0.0)
            nc.gpsimd.memset(qst[:, 1], 0.0)
            # one-time zero/one fills (pads persist across frames); off the
            # vector engine so the LN chain starts sooner
            for f in range(FPC):
                nc.gpsimd.memset(xnT[f][:, :, 0:32], 0.0)
                nc.gpsimd.memset(xnT[f][:, :, 32 + N:], 0.0)
                nc.gpsimd.memset(kTp[f][:, :, 0:32], 0.0)
                nc.gpsimd.memset(kTp[f][:, :, 32 + N:], 0.0)
                # vau: ones only where the v casts do not write -- the
                # denominator column 32 of every head, and the tail rows of
                # chunk 8. vau64 needs no fill: its two DMAs cover all of it.
                nc.gpsimd.memset(vau[f][:, :, :, 32], 1.0)
                nc.gpsimd.memset(vau[f][64:128, 8], 1.0)

            def pe_filler(n, dep=None):
                # HAM warm-keeper built from the on-chip identity so it never
                # waits on a DMA. A `dep` AP (read as the moving operand)
                # anchors the filler to pipeline progress -- without it the
                # scheduler bunches all fillers at t=0 and the PE then idles
                # long enough for HAM to re-throttle.
                for _ in range(n):
                    dmy = pst.tile([128, 2, 4, 64], dt.float32,
                                   tag="pst", name="dmy")
                    mv_ap = ident[:] if dep is None else dep
                    ncols = int(np.prod(mv_ap.shape[1:]))
                    nc.tensor.matmul(
                        dmy[:].rearrange("p q g j -> p (q g j)")[:, 0:ncols],
                        ident[:], mv_ap, start=True, stop=True)

            pe_filler(4)
            # x loads in 256-token chunks so the LN can start on chunk 0
            # while the rest streams; all on the sync ring, frame 0 first
            for f in range(FPC):
                for q4 in range(4):
                    eng = nc.sync
                    eng.dma_start(
                        x_f[f][:, 2 * q4:2 * q4 + 2, :],
                        x_d[f * N + 256 * q4:f * N + 256 * (q4 + 1), :]
                        .rearrange("(b p) d -> p b d", p=128))
                    if f == 0:
                        # fillers anchored to each arriving x chunk: they pace
                        # with the DMA and bridge the PE through the
                        # vector-bound LN phase without HAM re-throttling
                        pe_filler(2, dep=x_f[0][:, 2 * q4, 0:128])

            def ln_bn(f, i):
                st = work.tile([128, 6], dt.float32, tag="bnst", name="st")
                nc.vector.bn_stats(st[:], x_f[f][:, i, :])
                nc.vector.bn_aggr(mv[f][:, i, :], st[:])

            def ln_rstd(f, half):
                hs = slice(4 * half, 4 * half + 4)
                nc.scalar.activation(lnv[f][:, hs], mv[f][:, hs, 1],
                                     AF.Sqrt, bias=LN_EPS, scale=1.0)
                nc.vector.reciprocal(rstd[f][:, hs], lnv[f][:, hs])

            def ln_xnt(f, i, warm=False):
                xn = work.tile([128, 256], dt.bfloat16, tag="xn", name="xn")
                nc.vector.tensor_scalar(
                    xn[:], x_f[f][:, i, :], mv[f][:, i, 0:1], rstd[f][:, i:i + 1],
                    OP.subtract, OP.mult)
                if warm:
                    # anchored filler: keeps the PE array warm through the
                    # vector-bound LN phase, pacing with LN progress
                    pe_filler(2, dep=xn[:, 0:128])
                ptj = ptp.tile([128, 256], dt.bfloat16, tag="ptj", name="ptj")
                ptr = ptj[:, 0:256]
                for kc in range(2):
                    nc.tensor.transpose(
                        ptr[:, 128 * kc:128 * (kc + 1)],
                        xn[:, 128 * kc:128 * (kc + 1)], ident[:])
                nc.vector.tensor_copy(
                    xnT[f][:, :, 32 + 128 * i:32 + 128 * (i + 1)],
                    ptr[:].rearrange("p (k t) -> p k t", k=2))

            def qproj(f, nh):
                ns = slice(512 * nh, 512 * (nh + 1))
                for mc in range(2):
                    pq = pp.tile([128, 512], dt.float32, tag="pp")
                    for kc in range(2):
                        nc.tensor.matmul(
                            pq[:], wq_s[:, kc, 128 * mc:128 * (mc + 1)],
                            xnT[f][:, kc, 32 + 512 * nh:32 + 512 * (nh + 1)],
                            start=(kc == 0), stop=(kc == 1))
                    qn = work.tile([128, 512], dt.bfloat16, tag="qn")
                    # psum->sbuf copy with the q bias fused in (Identity
                    # activation takes a per-partition bias AP)
                    nc.scalar.activation(
                        qn[:], pq[:], AF.Identity,
                        bias=bqp_s[:, mc:mc + 1], scale=1.0)
                    # scatters stay off the sync ring: they are gated on qn
                    # and would head-of-line block the vau64 copies there
                    for g in range(4):
                        eng = nc.gpsimd if g % 2 == 0 else nc.scalar
                        eng.dma_start(
                            qst[32 * g:32 * (g + 1), mc, g, ns],
                            qn[32 * g:32 * (g + 1), :])

            def kproj(f, nh):
                for mc in range(2):
                    pk = pp.tile([128, 512], dt.float32, tag="pp")
                    for kc in range(2):
                        nc.tensor.matmul(
                            pk[:], wk_s[:, kc, 128 * mc:128 * (mc + 1)],
                            xnT[f][:, kc, 32 + 512 * nh:32 + 512 * (nh + 1)],
                            start=(kc == 0), stop=(kc == 1))
                    # split the psum->sbuf casts across vector and scalar so
                    # neither queue serializes the frame transition
                    dst = kTp[f][:, mc, 32 + 512 * nh:32 + 512 * (nh + 1)]
                    if mc == 0:
                        nc.vector.tensor_copy(dst, pk[:])
                    else:
                        nc.scalar.copy(dst, pk[:])

            def vproj(f):
                for c in range(9):
                    np_ = 128 if c < 8 else 64
                    pvv = pav.tile([128, NH, 33], dt.float32, tag="pav", name="pvv")
                    for kc in range(2):
                        nc.tensor.matmul(
                            pvv[0:np_, :, 0:32],
                            xnT[f][:, kc, 128 * c:128 * c + np_],
                            wv_s[:, kc, :],
                            start=(kc == 0), stop=(kc == 1))
                    nc.scalar.copy(vau[f][0:np_, c, :, 0:32], pvv[0:np_, :, 0:32])
                # hardware DGE ring (sync): the gpsimd software DGE is far too
                # slow for these partition-shifting copies and stalled the
                # first odd-subtile AV matmuls by ~2.4us. Split 4 ways so the
                # first halves (needed by AV of subtile 1) only depend on
                # v-chunks 0-4, not the whole projection.
                nc.sync.dma_start(vau64[f][0:64, 0:4], vau[f][64:128, 0:4])
                nc.sync.dma_start(vau64[f][64:128, 0:4], vau[f][0:64, 1:5])
                nc.sync.dma_start(vau64[f][0:64, 4:8], vau[f][64:128, 4:8])
                nc.sync.dma_start(vau64[f][64:128, 4:8], vau[f][0:64, 5:9])

            for f in range(FPC):
                if f == 0:
                    # LN half 0, then overlap LN half 1 (vector engine) with
                    # the first projections (tensor engine). All LN vector
                    # work is emitted before the kTp psum-copies: the vector
                    # queue is strict FIFO, so a psum-copy that waits on a
                    # projection matmul must not get ahead of LN chunks.
                    for i in range(4):
                        ln_bn(f, i)
                    ln_rstd(f, 0)
                    # late-needed const DMAs, emitted after the first Sqrt so
                    # their descriptor-gen fills scalar-queue idle instead of
                    # delaying the LN chain
                    nc.scalar.dma_start(bqp_s[:], bqp_d[:])
                    nc.scalar.dma_start(bob_s[:], bob_d[:])
                    nc.scalar.dma_start(lnm_s[:], lnm_d[:])
                    for i in range(4):
                        ln_xnt(f, i, warm=True)
                    for i in range(4, 8):
                        ln_bn(f, i)
                    ln_rstd(f, 1)
                    qproj(f, 0)
                    for i in range(4, 8):
                        ln_xnt(f, i)
                    qproj(f, 1)
                    kproj(f, 0)
                    kproj(f, 1)
                    vproj(f)
                    # frame 1's LN stats in the preamble's vector-engine
                    # slack (x frame 1 has landed by now); the normalize +
                    # transpose steps stay in the attention prewarm slots
                    for i in range(8):
                        ln_bn(1, i)
                    ln_rstd(1, 0)
                    ln_rstd(1, 1)
                else:
                    # frame 1's LN was pre-warmed inside frame 0's attention
                    qproj(f, 0)
                    qproj(f, 1)
                    kproj(f, 0)
                    kproj(f, 1)
                    vproj(f)
                pe_filler(2)

                # ---------------- attention: software-pipelined subtiles -----
                # slot t: scores(t) | AV(t-2) | pair-tail((t-5)//2)
                psts = {}
                aes = {}
                pas = {}

                def em_scores(s):
                    pst_t = pst.tile([128, 2, 4, 64], dt.float32, tag="pst", name="pst_t")
                    psts[s] = pst_t
                    vi = 0 if s == 0 else (2 if s == 15 else 1)
                    nc.tensor.matmul(
                        pst_t[:].rearrange("p q g j -> p (q g j)"),
                        ident8[:], lnm_s[:, vi, :], start=True, stop=False)
                    for Q in range(2):
                        nc.tensor.matmul(
                            pst_t[:, Q], kTp[f][:, Q, 64 * s:64 * s + 128],
                            qst[:, Q, :, 64 * s:64 * s + 64],
                            start=False, stop=(Q == 1), skip_group_check=True)

                def em_exp(s):
                    ae = att.tile([128, NH, 64], dt.bfloat16, tag="ae", name="ae")
                    aes[s] = ae
                    nc.scalar.activation(
                        ae[:], psts[s][:].rearrange("p q g j -> p (q g) j"),
                        AF.Exp, bias=0.0, scale=1.0)

                def em_av(s):
                    if s % 2 == 0:
                        pa = pav.tile([128, NH, 33], dt.float32, tag="pav", name="pa")
                        pas[s // 2] = pa
                    pa = pas[s // 2]
                    qs = slice(0, 64) if s % 2 == 0 else slice(64, 128)
                    vsrc = vau[f][:, s // 2] if s % 2 == 0 else vau64[f][:, (s - 1) // 2]
                    for h in range(NH):
                        nc.tensor.matmul(
                            pa[qs, h, :], aes[s][:, h, :], vsrc[:, h, :],
                            start=True, stop=True)

                ptjs = {}

                def em_pair_T(p):
                    pa = pas[p]
                    rc = att.tile([128, NH], dt.float32, tag="rc", name="rc")
                    nc.vector.reciprocal(rc[:], pa[:, :, 32])
                    on2 = att.tile([128, NH, 32], dt.bfloat16, tag="on2", name="on2")
                    nc.vector.tensor_tensor(
                        on2[:], pa[:, :, 0:32],
                        rc[:].unsqueeze(2).to_broadcast((128, NH, 32)), OP.mult)
                    on2v = on2[:].rearrange("p h c -> p (h c)")
                    ptj = ptp.tile([128, 256], dt.bfloat16, tag="ptj", name="ptj")
                    ptjs[p] = ptj
                    ptr = ptj[:, 0:256]
                    for kc in range(2):
                        nc.tensor.transpose(
                            ptr[:, 128 * kc:128 * (kc + 1)],
                            on2v[:, 128 * kc:128 * (kc + 1)], ident[:])
                    nc.vector.tensor_copy(
                        xoT[f][:, :, 128 * p:128 * (p + 1)],
                        ptr[:].rearrange("p (k t) -> p k t", k=2))

                def em_pair_O(p):
                    # out projection + residual (x_f already carries +bo) +
                    # store for token block p
                    pyt = pp.tile([128, 512], dt.float32, tag="pp", name="pyt")
                    py = pyt[:, 0:256]
                    for kc in range(2):
                        nc.tensor.matmul(
                            py[:], xoT[f][:, kc, 128 * p:128 * (p + 1)],
                            wo_s[:, kc, :], start=(kc == 0), stop=(kc == 1))
                    ys = work.tile([128, 256], dt.bfloat16, tag="ys", name="ys")
                    nc.vector.tensor_tensor(ys[:], py[:], x_f[f][:, p, :], OP.add)
                    nc.sync.dma_start(
                        y_d[f * N + 128 * p:f * N + 128 * (p + 1), :], ys[:])

                prewarm = {}
                if f == 0:
                    prewarm = {
                        5: [lambda: ln_xnt(1, 0)],
                        7: [lambda: ln_xnt(1, 1)],
                        9: [lambda: ln_xnt(1, 2)],
                        11: [lambda: ln_xnt(1, 3)],
                        13: [lambda: ln_xnt(1, 4)],
                        15: [lambda: ln_xnt(1, 5)],
                        17: [lambda: ln_xnt(1, 6)],
                        19: [lambda: ln_xnt(1, 7)],
                    }
                for t in range(23):
                    if t < 16:
                        em_scores(t)
                    # the output-proj bias folds into the residual here (not
                    # in ln_xnt): the gpsimd queue must first finish the qst
                    # scatter descriptor-gen that gates the first scores
                    if t in (1, 3):
                        for i in range(4 * (t // 2), 4 * (t // 2) + 4):
                            nc.gpsimd.tensor_tensor(
                                x_f[f][:, i, :], x_f[f][:, i, :], bob_s[:],
                                OP.add)
                    # tail slots need no fillers: the next frame's projection
                    # matmuls are already queued behind and keep the PE warm
                    if 1 <= t < 17:
                        em_exp(t - 1)
                    if 2 <= t < 18:
                        em_av(t - 2)
                    if t >= 5 and (t - 5) % 2 == 0 and (t - 5) // 2 < 8:
                        em_pair_T((t - 5) // 2)
                    if t >= 7 and (t - 7) % 2 == 0 and (t - 7) // 2 < 8:
                        em_pair_O((t - 7) // 2)
                    for fn in prewarm.get(t, ()):
                        fn()

    nc.compile()
    return nc


# ---------------------------------------------------------------- entry point
def kernel(**inputs):
    global _COMPILED
    if _COMPILED is None:
        _COMPILED = _build_bass()
    nc = _COMPILED

    from concourse.bass_utils import run_bass_kernel_spmd

    x = np.asarray(inputs["x"], dtype=np.float32)          # [2, 8, 32, 32, 256]
    B, T = x.shape[0], x.shape[1]
    frames = x.reshape(B * T, N, D).astype(ml_dtypes.bfloat16)
    params = _fold_params({k: np.asarray(v) for k, v in inputs.items()})

    in_maps = []
    for c in range(N_CORES):
        m = {"x": np.ascontiguousarray(
            frames[FPC * c:FPC * (c + 1)].reshape(FPC * N, D))}
        m.update(params)
        in_maps.append(m)

    res = run_bass_kernel_spmd(nc, in_maps, list(range(N_CORES)))
    y = np.concatenate([res.results[c]["y"].astype(np.float32).reshape(FPC, N, D)
                        for c in range(N_CORES)], axis=0)
    return y.reshape(x.shape).astype(np.float32)

